# revision 36
# baseline (speedup 1.0000x reference)
"""Causal self-attention on 8 Trainium2 cores.

Sharding: tensor-parallel over heads (4 groups of 4 heads) x data-parallel
over batch (2): each core computes q/k/v projections for its 4 heads, causal
attention, and a partial output projection through its slice of Wp's input
axis; the host sums the 4 partials per batch (the TP all-reduce) and adds the
output bias.

Per-core kernel layout (all-bf16 dataflow; psum accumulation stays f32):
- q,k are computed transposed (head-dim on partitions), the operand layout
  the S^T = K Q^T matmul wants.
- S is computed transposed (keys on partitions, queries on free dim), so
  P^T = exp(S^T) is exactly the *stationary* operand the PV matmul wants:
  y[q, d] = sum_k P^T[k, q] V[k, d] with V as the 65-wide moving operand
  (64 dims + a ones-column that accumulates the softmax denominator).
  Moving-free-size is what matmul costs, so PV costs 65 cycles per
  (query-tile, key-tile) instead of 512.
- y lands queries-on-partitions, so the softmax denominator is a
  per-partition scalar: reciprocal + tensor_scalar_mul on DVE, fused with
  the psum->sbuf eviction.
- y is then transposed 128x128 on the PE (bf16 transpose, 128 cycles) to
  feed the output projection, whose result lands back in [T, C] layout.
- exp() skips max-subtraction: logits are ~N(0,1) so overflow is impossible;
  the 1/sqrt(hd) scale is fused into the ACT op.
- The causal frontier block of P^T is zeroed with a 0/1 triangular mask
  multiply on the otherwise-idle gpsimd engine.
- Emission is head-staggered: head h's S/exp stream is interleaved with head
  h-1's PV groups plus projection / v / output-projection / transpose work,
  so the in-order PE queue never waits on ACT's exp latency.
"""
import sys
import numpy as np

sys.path.insert(0, "/opt/trn_rl_repo")

import concourse.bass as bass  # noqa: E402
import concourse.mybir as mybir  # noqa: E402
import concourse.tile as tile  # noqa: E402
from concourse import bacc  # noqa: E402
from concourse.bass_utils import run_bass_kernel_spmd  # noqa: E402

B, T, C, H = 2, 2048, 1024, 16
HD = C // H            # 64 head dim
GROUPS = 4             # head groups (tensor-parallel degree)
HPG = H // GROUPS      # 4 heads per group
OS = HPG * HD          # 256 = per-core qkv output slice
N_CORES = B * GROUPS   # 8
TCH = 512              # query chunk (psum free width)
NT = T // 128          # 16 key tiles
NCH = T // TCH         # 4 query chunks
KC = C // 128          # 8 contraction tiles for projections
VW = HPG * (HD + 1)    # 260: v with interleaved ones-columns

F32 = mybir.dt.float32
BF16 = mybir.dt.bfloat16
F8 = mybir.dt.float8e4

# wqk dram column offset per m (m 0,1 = q halves; 2,3 = k halves); m0/m2
# loaded first so heads 0,1 can start while m1/m3 are still in flight
WQK_OFF = {0: 0, 2: 1024, 1: 2048, 3: 3072}

_CACHE = {}
PHASES = []  # (label, next-instr-name) emission marks, for dev profiling


def _build():
    nc = bacc.Bacc("TRN2", target_bir_lowering=False, debug=False)

    xT = nc.declare_dram_parameter("xT", [C, T], BF16, isOutput=False)
    wqk = nc.declare_dram_parameter("wqk", [128, 4 * KC * 128], BF16,
                                    isOutput=False)
    wv = nc.declare_dram_parameter("wv", [128, KC * VW], BF16, isOutput=False)
    wp = nc.declare_dram_parameter("wp", [128, 2 * C], BF16, isOutput=False)
    smf = nc.declare_dram_parameter("smf", [128, 4], F32, isOutput=False)
    # cols 0:128 tri mask, 128:256 identity, row0 256:516 bv_aug,
    # row0 516:644 ones
    smb = nc.declare_dram_parameter("smb", [128, 644], BF16, isOutput=False)
    out = nc.declare_dram_parameter("out", [T, C], BF16, isOutput=True)

    Exp = mybir.ActivationFunctionType.Exp

    with tile.TileContext(nc) as tc:
        with (
            nc.allow_low_precision(reason="bf16 dataflow, tol 2e-2"),
            tc.tile_pool(name="xt", bufs=1) as xt_pool,
            tc.tile_pool(name="wts", bufs=1) as w_pool,
            tc.tile_pool(name="qk", bufs=1) as qk_pool,
            tc.tile_pool(name="vsb", bufs=1) as v_pool,
            tc.tile_pool(name="yt", bufs=1) as yt_pool,
            tc.tile_pool(name="pt", bufs=26) as pt_pool,
            tc.tile_pool(name="ysb", bufs=10) as y_pool,
            tc.tile_pool(name="rcp", bufs=4) as rcp_pool,
            tc.tile_pool(name="osb", bufs=3) as ot_pool,
            tc.tile_pool(name="psm", bufs=2, space="PSUM") as ps_main,
            tc.tile_pool(name="pss", bufs=2, space="PSUM") as ps_s,
            tc.tile_pool(name="psy", bufs=2, space="PSUM") as ps_y,
        ):
            # ---- SBUF tiles ----
            xt_b = xt_pool.tile([128, KC * T], BF16, tag="xtb", name="xtb")
            xt = [xt_b[:, k * T:(k + 1) * T] for k in range(KC)]
            wqk_b = w_pool.tile([128, 4 * KC * 128], BF16, tag="wqkb",
                                name="wqkb")
            wqk_t = {m: wqk_b[:, WQK_OFF[m]:WQK_OFF[m] + KC * 128]
                     for m in range(4)}
            wv_b = w_pool.tile([128, KC * VW], BF16, tag="wvb", name="wvb")
            wv_t = [wv_b[:, k * VW:(k + 1) * VW] for k in range(KC)]
            wp_b = w_pool.tile([128, 2 * C], BF16, tag="wpb", name="wpb")
            wp_t = [wp_b[:, k * C:(k + 1) * C] for k in range(2)]
            smf_b = w_pool.tile([128, 4], F32, tag="smfb", name="smfb")
            bqk_t = [smf_b[:, m:m + 1] for m in range(4)]
            smb_b = w_pool.tile([128, 644], BF16, tag="smbb", name="smbb")
            tri_t = smb_b[:, 0:128]
            ident_t = smb_b[:, 128:256]
            bv_t = smb_b[0:1, 256:256 + VW]
            ones_t = smb_b[0:1, 516:644]

            qk_sb = [qk_pool.tile([128, T], BF16, tag=f"qk{m}", name=f"qk{m}")
                     for m in range(4)]
            v_sb = [v_pool.tile([128, VW], BF16, tag=f"v{i}", name=f"v{i}")
                    for i in range(NT)]
            # yt: per query-tile i, cols i*256 + k*128 hold y^T d-half k
            yt_sb = yt_pool.tile([128, NT * 256], BF16, tag="ytb", name="ytb")

            # ---- input DMAs (transfers drain serially in emission order) ----
            xt_v = xt_b[:].rearrange("p (k t) -> p k t", k=KC)
            xT_v = xT[:, :].rearrange("(k p) t -> p k t", p=128)
            NSL = 8
            SL = T // NSL

            def slab(d):
                nc.sync.dma_start(xt_v[:, :, d * SL:(d + 1) * SL],
                                  xT_v[:, :, d * SL:(d + 1) * SL])

            nc.sync.dma_start(wqk_b[:, 0:1024], wqk[:, 0:1024])    # m0
            slab(0)
            nc.sync.dma_start(wqk_b[:, 1024:2048], wqk[:, 1024:2048])  # m2
            slab(1)
            nc.sync.dma_start(smf_b[:], smf[:])
            nc.sync.dma_start(smb_b[:], smb[:])
            nc.sync.dma_start(wv_b[:], wv[:])
            slab(2)
            nc.sync.dma_start(wqk_b[:, 2048:4096], wqk[:, 2048:4096])
            slab(3)
            nc.sync.dma_start(wp_b[:], wp[:])
            for d in range(4, NSL):
                slab(d)

            # ---- emission helpers ----
            pt_tiles = {}   # (h, j) -> (P^T pair tile, col base)
            y_tiles = {}    # qt -> y [128 queries, 256 dims] sbuf tile
            ot_tiles = {}   # qt -> out staging tile

            def mark(label):
                PHASES.append((label, nc.get_next_instruction_name()))

            def do_proj(m, cch, halves=((0, TCH),)):
                mark(f"proj{m}.{cch}")
                c0 = cch * TCH
                for h0, hw in halves:
                    ps = ps_main.tile([128, TCH], F32, tag="pmain",
                                      name="pmain")
                    for k in range(KC):
                        nc.tensor.matmul(
                            ps[:, 0:hw],
                            wqk_t[m][:, k * 128:(k + 1) * 128],
                            xt[k][:, c0 + h0:c0 + h0 + hw],
                            start=(k == 0),
                            stop=(k == KC - 1),
                        )
                    nc.vector.tensor_scalar_add(
                        qk_sb[m][:, c0 + h0:c0 + h0 + hw], ps[:, 0:hw],
                        bqk_t[m][:])

            def do_v(i):
                mark(f"v.{i}")
                ps = ps_main.tile([128, VW], F32, tag="pmain", name="pmain")
                for k in range(KC):
                    nc.tensor.matmul(
                        ps[:],
                        xt[k][:, i * 128:(i + 1) * 128],
                        wv_t[k][:],
                        start=(k == 0),
                        stop=False,
                    )
                # rank-1 bias add: ones^T @ bv_aug (also writes the 1.0s)
                nc.tensor.matmul(ps[:], ones_t[:], bv_t[:],
                                 start=False, stop=True)
                nc.vector.tensor_copy(v_sb[i][:], ps[:])

            def do_S_pair(h, cch, jp):
                # two key tiles j0,j1 into one 2-bank psum pair; full tiles
                # share one exp instruction to amortize ACT access overhead,
                # diagonal pairs merge into regular-strided exps
                mark(f"S{h}.{cch}.{jp}")
                c0, c1 = cch * TCH, (cch + 1) * TCH
                qrow = (h % 2) * 64
                qm, km = h // 2, 2 + h // 2
                pss = ps_s.tile([128, 2 * TCH], F32, tag="ps", name="ps")
                pt = pt_pool.tile([128, 2 * TCH], BF16, tag="pt", name="pt")
                los = []
                for half in range(2):
                    j = 2 * jp + half
                    r = j - 4 * cch
                    lo = 128 * r if r > 0 else 0
                    los.append((j, r, lo, half * TCH))
                    nc.tensor.matmul(
                        pss[:, half * TCH + lo:(half + 1) * TCH],
                        qk_sb[km][qrow:qrow + 64, j * 128:(j + 1) * 128],
                        qk_sb[qm][qrow:qrow + 64, c0 + lo:c1],
                        start=True,
                        stop=True,
                    )
                    pt_tiles[(h, j)] = (pt, half * TCH)
                scale = 1.0 / np.sqrt(HD)
                if los[1][1] < 0:
                    # both tiles full: single fused exp over the pair
                    nc.scalar.activation(pt[:], pss[:], Exp, scale=scale)
                else:
                    for j, r, lo, base in los:
                        nc.scalar.activation(
                            pt[:, base + lo:base + TCH],
                            pss[:, base + lo:base + TCH],
                            Exp, scale=scale)
                        if r >= 0:
                            # zero the causal-frontier block (0/1 tri mask)
                            nc.gpsimd.tensor_mul(
                                pt[:, base + lo:base + lo + 128],
                                pt[:, base + lo:base + lo + 128], tri_t[:])

            def do_pv(h, i):
                # y_aug[q, 0:65] for query tile i, head h: P^T stationary
                mark(f"pv{h}.{i}")
                qt = i % 4
                py = ps_y.tile([128, HD + 1], F32, tag="py", name="py")
                for j in range(i + 1):
                    pt, base = pt_tiles[(h, j)]
                    nc.tensor.matmul(
                        py[:],
                        pt[:, base + qt * 128:base + (qt + 1) * 128],
                        v_sb[j][:, h * (HD + 1):(h + 1) * (HD + 1)],
                        start=(j == 0),
                        stop=(j == i),
                    )
                # normalize while evicting: y = y_aug[:, 0:64] / denom
                if h == 0:
                    y_tiles[i] = y_pool.tile([128, 256], BF16, tag="y",
                                             name="y")
                rcp = rcp_pool.tile([128, 1], F32, tag="rcp", name="rcp")
                nc.vector.reciprocal(rcp[:], py[:, HD:HD + 1])
                nc.vector.tensor_scalar_mul(
                    y_tiles[i][:, h * HD:(h + 1) * HD], py[:, 0:HD], rcp[:])

            def do_T(i):
                # y [q, d] -> y^T [d, q] via the DMA XBAR transpose
                # (sbuf->sbuf, 2-byte dtype): no PE, DVE, or PSUM involved
                mark(f"T.{i}")
                for k in range(2):
                    nc.sync.dma_start(
                        yt_sb[:, i * 256 + k * 128:i * 256 + (k + 1) * 128],
                        y_tiles[i][:, k * 128:(k + 1) * 128],
                        transpose=True)
                del y_tiles[i]

            def do_oproj(i, o):
                mark(f"op{o}.{i}")
                ps = ps_main.tile([128, TCH], F32, tag="pmain", name="pmain")
                for k in range(2):
                    nc.tensor.matmul(
                        ps[:],
                        yt_sb[:, i * 256 + k * 128:i * 256 + (k + 1) * 128],
                        wp_t[k][:, o * TCH:(o + 1) * TCH],
                        start=(k == 0),
                        stop=(k == 1),
                    )
                if o == 0:
                    ot_tiles[i] = ot_pool.tile([128, 2 * TCH], BF16, tag="ot",
                                               name="ot")
                nc.vector.tensor_copy(
                    ot_tiles[i][:, o * TCH:(o + 1) * TCH], ps[:])
                if o == 1:
                    nc.sync.dma_start(out[i * 128:(i + 1) * 128, :],
                                      ot_tiles.pop(i)[:])

            # ---- schedule ----
            def run_slot(h, cch, fillers, pre=0):
                """Emit head h's S/exp stream for chunk cch, spreading the
                filler thunks between S steps so the in-order PE queue always
                has ready work while ACT computes exp. The first `pre`
                fillers go ahead of the first S pair (covering repack/evict
                latency at slot starts)."""
                NP = 2 * cch + 2
                done = 0
                while done < pre:
                    fillers[done]()
                    done += 1
                for jp in range(NP):
                    do_S_pair(h, cch, jp)
                    want = max(done, (len(fillers) * (jp + 1)) // NP)
                    while done < want:
                        fillers[done]()
                        done += 1

            # PV(h) of chunk c is emitted two slots after its S stream:
            # pv0 -> slot2(c), pv1 -> slot3(c), pv2 -> slot0(c+1),
            # pv3 -> slot1(c+1); transposes + output projection for chunk c
            # land in slots 2/3 of chunk c+1. This balances filler work
            # across slots so no slot's PE stream is thinner than its ACT
            # exp load, and gives every PE consumer >= 1us of slack after
            # its DVE/DMA producer.
            for cch in range(NCH):
                i0 = 4 * cch
                p0 = i0 - 4  # prev chunk query-tile base
                last = cch == NCH - 1
                if cch == 0:
                    # first chunk chases the xT slab arrivals: 256-query
                    # projection halves per slab
                    do_proj(0, 0, halves=((0, 256), (256, 256)))
                    do_proj(2, 0, halves=((0, 256), (256, 256)))
                    s0 = [lambda i=i: do_v(i) for i in range(4)]
                    s1 = [lambda: do_proj(1, 0, halves=((0, 256),)),
                          lambda: do_proj(1, 0, halves=((256, 256),)),
                          lambda: do_proj(3, 0, halves=((0, 256),)),
                          lambda: do_proj(3, 0, halves=((256, 256),))]
                    s2 = [lambda i=i: do_pv(0, i) for i in range(4)]
                    s3 = [lambda i=i: do_pv(1, i) for i in range(4)]
                    pres = (0, 0, 0, 0)
                else:
                    do_proj(0, cch)
                    do_proj(2, cch)
                    s0 = [lambda: do_pv(2, p0 + 0),
                          lambda: do_v(i0 + 0),
                          lambda: do_pv(2, p0 + 1),
                          lambda: do_v(i0 + 1),
                          lambda: do_pv(2, p0 + 2),
                          lambda: do_v(i0 + 2),
                          lambda: do_pv(2, p0 + 3),
                          lambda: do_v(i0 + 3)]
                    s1 = [lambda: do_proj(1, cch),
                          lambda: do_pv(3, p0 + 0),
                          lambda: do_pv(3, p0 + 1),
                          lambda: do_proj(3, cch),
                          lambda: do_pv(3, p0 + 2),
                          lambda: do_pv(3, p0 + 3)]
                    s2 = [lambda: do_pv(0, i0 + 0),
                          lambda: do_T(p0 + 0),
                          lambda: do_pv(0, i0 + 1),
                          lambda: do_T(p0 + 1),
                          lambda: do_oproj(p0 + 0, 0),
                          lambda: do_pv(0, i0 + 2),
                          lambda: do_T(p0 + 2),
                          lambda: do_oproj(p0 + 0, 1),
                          lambda: do_pv(0, i0 + 3),
                          lambda: do_oproj(p0 + 1, 0),
                          lambda: do_T(p0 + 3),
                          lambda: do_oproj(p0 + 1, 1)]
                    s3 = [lambda: do_pv(1, i0 + 0),
                          lambda: do_oproj(p0 + 2, 0),
                          lambda: do_pv(1, i0 + 1),
                          lambda: do_oproj(p0 + 2, 1),
                          lambda: do_pv(1, i0 + 2),
                          lambda: do_oproj(p0 + 3, 0),
                          lambda: do_pv(1, i0 + 3),
                          lambda: do_oproj(p0 + 3, 1)]
                    if last:
                        # densify the final slot so the drain is shorter:
                        # head-2 PV for the first query tiles only needs
                        # slot-2's S pairs, which are already emitted
                        s3 = s3 + [lambda: do_pv(2, i0 + 0),
                                   lambda: do_pv(2, i0 + 1)]
                    pres = (0, 0, 0, 0)
                run_slot(0, cch, s0, pre=pres[0])
                run_slot(1, cch, s1, pre=pres[1])
                run_slot(2, cch, s2, pre=pres[2])
                run_slot(3, cch, s3, pre=pres[3])

            # drain: chunk 3's heads 2,3 PV, transposes, output projection
            i0 = 4 * (NCH - 1)
            for u in [
                lambda: do_pv(2, i0 + 2),
                lambda: do_pv(2, i0 + 3),
                lambda: do_pv(3, i0 + 0),
                lambda: do_pv(3, i0 + 1),
                lambda: do_T(i0 + 0),
                lambda: do_pv(3, i0 + 2),
                lambda: do_T(i0 + 1),
                lambda: do_oproj(i0 + 0, 0),
                lambda: do_pv(3, i0 + 3),
                lambda: do_T(i0 + 2),
                lambda: do_oproj(i0 + 0, 1),
                lambda: do_oproj(i0 + 1, 0),
                lambda: do_T(i0 + 3),
                lambda: do_oproj(i0 + 1, 1),
                lambda: do_oproj(i0 + 2, 0),
                lambda: do_oproj(i0 + 2, 1),
                lambda: do_oproj(i0 + 3, 0),
                lambda: do_oproj(i0 + 3, 1),
            ]:
                u()

    nc.compile()
    return nc


def _host_inputs(x, Wq, bq, Wk, bk, Wv, bv, Wp):
    """Slice + lay out per-core inputs (bf16 except the f32 biases)."""
    import ml_dtypes
    BF = ml_dtypes.bfloat16

    t2l = np.arange(128)[:, None]
    bl = np.arange(128)[None, :]
    tri = (t2l <= bl).astype(np.float32)  # [keys, queries] causal keep-mask

    def fold(a):
        # (kc*128, w) -> (128, kc*w): k-tile index moves into the free dim
        kc, w = a.shape[0] // 128, a.shape[1]
        return np.ascontiguousarray(
            a.reshape(kc, 128, w).transpose(1, 0, 2).reshape(128, kc * w))

    xTs = [np.ascontiguousarray(x[b].T).astype(BF) for b in range(B)]
    grp = []
    for g in range(GROUPS):
        hs = g * OS
        he = hs + OS
        # m-major wqk: column blocks in load order m0, m2, m1, m3
        wq_T = Wq[hs:he].T  # [C, 256]
        wk_T = Wk[hs:he].T
        mslab = {0: wq_T[:, 0:128], 1: wq_T[:, 128:256],
                 2: wk_T[:, 0:128], 3: wk_T[:, 128:256]}
        wqk_s = np.zeros((128, 4 * KC * 128), dtype=np.float32)
        for m in range(4):
            wqk_s[:, WQK_OFF[m]:WQK_OFF[m] + KC * 128] = fold(mslab[m])
        bqk = np.stack([bq[hs:hs + 128], bq[hs + 128:he],
                        bk[hs:hs + 128], bk[hs + 128:he]], axis=1)
        wv_aug = np.zeros((C, VW), dtype=np.float32)
        bv_aug = np.zeros((VW,), dtype=np.float32)
        for h in range(HPG):
            lo = h * (HD + 1)
            wv_aug[:, lo:lo + HD] = Wv[hs + h * HD:hs + (h + 1) * HD].T
            bv_aug[lo:lo + HD] = bv[hs + h * HD:hs + (h + 1) * HD]
            bv_aug[lo + HD] = 1.0
        wp_s = fold(np.ascontiguousarray(Wp[:, hs:he].T))
        smb = np.zeros((128, 644), dtype=np.float32)
        smb[:, 0:128] = tri
        smb[:, 128:256] = np.eye(128, dtype=np.float32)
        smb[0, 256:256 + VW] = bv_aug
        smb[0, 516:644] = 1.0
        grp.append({"wqk": wqk_s.astype(BF), "wv": fold(wv_aug).astype(BF),
                    "wp": wp_s.astype(BF), "smf": bqk.astype(np.float32),
                    "smb": smb.astype(BF)})

    in_maps = []
    for ci in range(N_CORES):
        b, g = divmod(ci, GROUPS)
        in_maps.append({"xT": xTs[b], **grp[g]})
    return in_maps


def kernel(x, Wq, bq, Wk, bk, Wv, bv, Wp, bp):
    x = np.asarray(x, dtype=np.float32)
    args = [np.asarray(a, dtype=np.float32)
            for a in (Wq, bq, Wk, bk, Wv, bv, Wp)]
    bp = np.asarray(bp, dtype=np.float32)

    if "nc" not in _CACHE:
        _CACHE["nc"] = _build()
    nc = _CACHE["nc"]

    in_maps = _host_inputs(x, *args)
    res = run_bass_kernel_spmd(nc, in_maps, list(range(N_CORES)))

    out = np.empty((B, T, C), dtype=np.float32)
    for b in range(B):
        acc = np.asarray(res.results[b * GROUPS]["out"], dtype=np.float32)
        for g in range(1, GROUPS):
            acc += np.asarray(res.results[b * GROUPS + g]["out"],
                              dtype=np.float32)
        out[b] = acc + bp
    return out


# revision 50
# speedup vs baseline: 1.0058x; 1.0058x over previous
"""Causal self-attention on 8 Trainium2 cores.

Sharding: tensor-parallel over heads (4 groups of 4 heads) x data-parallel
over batch (2): each core computes q/k/v projections for its 4 heads, causal
attention, and a partial output projection through its slice of Wp's input
axis; the host sums the 4 partials per batch (the TP all-reduce) and adds the
output bias.

Per-core kernel layout (all-bf16 dataflow; psum accumulation stays f32):
- q,k are computed transposed (head-dim on partitions), the operand layout
  the S^T = K Q^T matmul wants.
- S is computed transposed (keys on partitions, queries on free dim), so
  P^T = exp(S^T) is exactly the *stationary* operand the PV matmul wants:
  y[q, d] = sum_k P^T[k, q] V[k, d] with V as the 65-wide moving operand
  (64 dims + a ones-column that accumulates the softmax denominator).
  Moving-free-size is what matmul costs, so PV costs 65 cycles per
  (query-tile, key-tile) instead of 512.
- y lands queries-on-partitions, so the softmax denominator is a
  per-partition scalar: reciprocal + tensor_scalar_mul on DVE, fused with
  the psum->sbuf eviction.
- y is then transposed 128x128 by the DMA XBAR (sbuf->sbuf, 2-byte dtype)
  to feed the output projection, whose result lands back in [T, C] layout
  without touching PE, DVE, or PSUM.
- S psum tiles come in 2-bank pairs: two key tiles share one exp
  instruction, amortizing the ACT engine's SBUF-access overhead.
- exp() skips max-subtraction: logits are ~N(0,1) so overflow is impossible;
  the 1/sqrt(hd) scale is fused into the ACT op.
- The causal frontier block of P^T is zeroed with a 0/1 triangular mask
  multiply on the otherwise-idle gpsimd engine.
- Emission is head-staggered: head h's S/exp stream is interleaved with head
  h-1's PV groups plus projection / v / output-projection / transpose work,
  so the in-order PE queue never waits on ACT's exp latency.
"""
import sys
import numpy as np

sys.path.insert(0, "/opt/trn_rl_repo")

import concourse.bass as bass  # noqa: E402
import concourse.mybir as mybir  # noqa: E402
import concourse.tile as tile  # noqa: E402
from concourse import bacc  # noqa: E402
from concourse.bass_utils import run_bass_kernel_spmd  # noqa: E402

B, T, C, H = 2, 2048, 1024, 16
HD = C // H            # 64 head dim
GROUPS = 4             # head groups (tensor-parallel degree)
HPG = H // GROUPS      # 4 heads per group
OS = HPG * HD          # 256 = per-core qkv output slice
N_CORES = B * GROUPS   # 8
TCH = 512              # query chunk (psum free width)
NT = T // 128          # 16 key tiles
NCH = T // TCH         # 4 query chunks
KC = C // 128          # 8 contraction tiles for projections
VW = HPG * (HD + 1)    # 260: v with interleaved ones-columns

F32 = mybir.dt.float32
BF16 = mybir.dt.bfloat16
F8 = mybir.dt.float8e4

# wqk dram column offset per m (m 0,1 = q halves; 2,3 = k halves); m0/m2
# loaded first so heads 0,1 can start while m1/m3 are still in flight
WQK_OFF = {0: 0, 2: 1024, 1: 2048, 3: 3072}

_CACHE = {}
PHASES = []  # (label, next-instr-name) emission marks, for dev profiling


def _build():
    nc = bacc.Bacc("TRN2", target_bir_lowering=False, debug=False)

    xT = nc.declare_dram_parameter("xT", [C, T], BF16, isOutput=False)
    wqk = nc.declare_dram_parameter("wqk", [128, 4 * KC * 128], BF16,
                                    isOutput=False)
    wv = nc.declare_dram_parameter("wv", [128, KC * VW], BF16, isOutput=False)
    wp = nc.declare_dram_parameter("wp", [128, 2 * C], BF16, isOutput=False)
    smf = nc.declare_dram_parameter("smf", [128, 4], F32, isOutput=False)
    # cols 0:128 tri mask, 128:256 identity, row0 256:516 bv_aug,
    # row0 516:644 ones
    smb = nc.declare_dram_parameter("smb", [128, 644], BF16, isOutput=False)
    out = nc.declare_dram_parameter("out", [T, C], BF16, isOutput=True)

    Exp = mybir.ActivationFunctionType.Exp

    with tile.TileContext(nc) as tc:
        with (
            nc.allow_low_precision(reason="bf16 dataflow, tol 2e-2"),
            tc.tile_pool(name="xt", bufs=1) as xt_pool,
            tc.tile_pool(name="wts", bufs=1) as w_pool,
            tc.tile_pool(name="qk", bufs=1) as qk_pool,
            tc.tile_pool(name="vsb", bufs=1) as v_pool,
            tc.tile_pool(name="yt", bufs=1) as yt_pool,
            tc.tile_pool(name="pt", bufs=28) as pt_pool,
            tc.tile_pool(name="ysb", bufs=12) as y_pool,
            tc.tile_pool(name="rcp", bufs=8) as rcp_pool,
            tc.tile_pool(name="osb", bufs=5) as ot_pool,
            tc.tile_pool(name="psm", bufs=2, space="PSUM") as ps_main,
            tc.tile_pool(name="pss", bufs=2, space="PSUM") as ps_s,
            tc.tile_pool(name="psy", bufs=2, space="PSUM") as ps_y,
        ):
            # ---- SBUF tiles ----
            xt_b = xt_pool.tile([128, KC * T], BF16, tag="xtb", name="xtb")
            xt = [xt_b[:, k * T:(k + 1) * T] for k in range(KC)]
            wqk_b = w_pool.tile([128, 4 * KC * 128], BF16, tag="wqkb",
                                name="wqkb")
            wqk_t = {m: wqk_b[:, WQK_OFF[m]:WQK_OFF[m] + KC * 128]
                     for m in range(4)}
            wv_b = w_pool.tile([128, KC * VW], BF16, tag="wvb", name="wvb")
            wv_t = [wv_b[:, k * VW:(k + 1) * VW] for k in range(KC)]
            wp_b = w_pool.tile([128, 2 * C], BF16, tag="wpb", name="wpb")
            wp_t = [wp_b[:, k * C:(k + 1) * C] for k in range(2)]
            smf_b = w_pool.tile([128, 4], F32, tag="smfb", name="smfb")
            bqk_t = [smf_b[:, m:m + 1] for m in range(4)]
            smb_b = w_pool.tile([128, 644], BF16, tag="smbb", name="smbb")
            tri_t = smb_b[:, 0:128]
            ident_t = smb_b[:, 128:256]
            bv_t = smb_b[0:1, 256:256 + VW]
            ones_t = smb_b[0:1, 516:644]

            qk_sb = [qk_pool.tile([128, T], BF16, tag=f"qk{m}", name=f"qk{m}")
                     for m in range(4)]
            v_sb = [v_pool.tile([128, VW], BF16, tag=f"v{i}", name=f"v{i}")
                    for i in range(NT)]
            # yt: per query-tile i, cols i*256 + k*128 hold y^T d-half k
            yt_sb = yt_pool.tile([128, NT * 256], BF16, tag="ytb", name="ytb")

            # ---- input DMAs (transfers drain serially in emission order) ----
            xt_v = xt_b[:].rearrange("p (k t) -> p k t", k=KC)
            xT_v = xT[:, :].rearrange("(k p) t -> p k t", p=128)
            NSL = 8
            SL = T // NSL

            def slab(d):
                nc.sync.dma_start(xt_v[:, :, d * SL:(d + 1) * SL],
                                  xT_v[:, :, d * SL:(d + 1) * SL])

            nc.sync.dma_start(wqk_b[:, 0:1024], wqk[:, 0:1024])    # m0
            slab(0)
            nc.sync.dma_start(wqk_b[:, 1024:2048], wqk[:, 1024:2048])  # m2
            slab(1)
            nc.sync.dma_start(smf_b[:], smf[:])
            nc.sync.dma_start(smb_b[:], smb[:])
            nc.sync.dma_start(wv_b[:], wv[:])
            slab(2)
            nc.sync.dma_start(wqk_b[:, 2048:4096], wqk[:, 2048:4096])
            slab(3)
            nc.sync.dma_start(wp_b[:], wp[:])
            for d in range(4, NSL):
                slab(d)

            # ---- emission helpers ----
            pt_tiles = {}   # (h, j) -> (P^T pair tile, col base)
            y_tiles = {}    # qt -> y [128 queries, 256 dims] sbuf tile
            ot_tiles = {}   # qt -> out staging tile

            def mark(label):
                PHASES.append((label, nc.get_next_instruction_name()))

            def do_proj(m, cch, halves=((0, TCH),)):
                mark(f"proj{m}.{cch}")
                c0 = cch * TCH
                for h0, hw in halves:
                    ps = ps_main.tile([128, TCH], F32, tag="pmain",
                                      name="pmain")
                    for k in range(KC):
                        nc.tensor.matmul(
                            ps[:, 0:hw],
                            wqk_t[m][:, k * 128:(k + 1) * 128],
                            xt[k][:, c0 + h0:c0 + h0 + hw],
                            start=(k == 0),
                            stop=(k == KC - 1),
                        )
                    nc.vector.tensor_scalar_add(
                        qk_sb[m][:, c0 + h0:c0 + h0 + hw], ps[:, 0:hw],
                        bqk_t[m][:])

            def do_v(i):
                mark(f"v.{i}")
                ps = ps_main.tile([128, VW], F32, tag="pmain", name="pmain")
                for k in range(KC):
                    nc.tensor.matmul(
                        ps[:],
                        xt[k][:, i * 128:(i + 1) * 128],
                        wv_t[k][:],
                        start=(k == 0),
                        stop=False,
                    )
                # rank-1 bias add: ones^T @ bv_aug (also writes the 1.0s)
                nc.tensor.matmul(ps[:], ones_t[:], bv_t[:],
                                 start=False, stop=True)
                nc.vector.tensor_copy(v_sb[i][:], ps[:])

            def do_S_pair(h, cch, jp):
                # two key tiles j0,j1 into one 2-bank psum pair; full tiles
                # share one exp instruction to amortize ACT access overhead,
                # diagonal pairs merge into regular-strided exps
                mark(f"S{h}.{cch}.{jp}")
                c0, c1 = cch * TCH, (cch + 1) * TCH
                qrow = (h % 2) * 64
                qm, km = h // 2, 2 + h // 2
                pss = ps_s.tile([128, 2 * TCH], F32, tag="ps", name="ps")
                pt = pt_pool.tile([128, 2 * TCH], BF16, tag="pt", name="pt")
                los = []
                for half in range(2):
                    j = 2 * jp + half
                    r = j - 4 * cch
                    lo = 128 * r if r > 0 else 0
                    los.append((j, r, lo, half * TCH))
                    nc.tensor.matmul(
                        pss[:, half * TCH + lo:(half + 1) * TCH],
                        qk_sb[km][qrow:qrow + 64, j * 128:(j + 1) * 128],
                        qk_sb[qm][qrow:qrow + 64, c0 + lo:c1],
                        start=True,
                        stop=True,
                    )
                    pt_tiles[(h, j)] = (pt, half * TCH)
                scale = 1.0 / np.sqrt(HD)
                if los[1][1] < 0:
                    # both tiles full: single fused exp over the pair
                    nc.scalar.activation(pt[:], pss[:], Exp, scale=scale)
                else:
                    for j, r, lo, base in los:
                        nc.scalar.activation(
                            pt[:, base + lo:base + TCH],
                            pss[:, base + lo:base + TCH],
                            Exp, scale=scale)
                        if r >= 0:
                            # zero the causal-frontier block (0/1 tri mask)
                            nc.gpsimd.tensor_mul(
                                pt[:, base + lo:base + lo + 128],
                                pt[:, base + lo:base + lo + 128], tri_t[:])

            def do_pv(h, i):
                # y_aug[q, 0:65] for query tile i, head h: P^T stationary
                mark(f"pv{h}.{i}")
                qt = i % 4
                py = ps_y.tile([128, HD + 1], F32, tag="py", name="py")
                for j in range(i + 1):
                    pt, base = pt_tiles[(h, j)]
                    nc.tensor.matmul(
                        py[:],
                        pt[:, base + qt * 128:base + (qt + 1) * 128],
                        v_sb[j][:, h * (HD + 1):(h + 1) * (HD + 1)],
                        start=(j == 0),
                        stop=(j == i),
                    )
                # normalize while evicting: y = y_aug[:, 0:64] / denom
                if h == 0:
                    y_tiles[i] = y_pool.tile([128, 256], BF16, tag="y",
                                             name="y")
                rcp = rcp_pool.tile([128, 1], F32, tag="rcp", name="rcp")
                nc.vector.reciprocal(rcp[:], py[:, HD:HD + 1])
                nc.vector.tensor_scalar_mul(
                    y_tiles[i][:, h * HD:(h + 1) * HD], py[:, 0:HD], rcp[:])

            def do_T(i):
                # y [q, d] -> y^T [d, q] via the DMA XBAR transpose
                # (sbuf->sbuf, 2-byte dtype): no PE, DVE, or PSUM involved
                mark(f"T.{i}")
                for k in range(2):
                    nc.sync.dma_start(
                        yt_sb[:, i * 256 + k * 128:i * 256 + (k + 1) * 128],
                        y_tiles[i][:, k * 128:(k + 1) * 128],
                        transpose=True)
                del y_tiles[i]

            def do_oproj(i, o):
                mark(f"op{o}.{i}")
                ps = ps_main.tile([128, TCH], F32, tag="pmain", name="pmain")
                for k in range(2):
                    nc.tensor.matmul(
                        ps[:],
                        yt_sb[:, i * 256 + k * 128:i * 256 + (k + 1) * 128],
                        wp_t[k][:, o * TCH:(o + 1) * TCH],
                        start=(k == 0),
                        stop=(k == 1),
                    )
                if o == 0:
                    ot_tiles[i] = ot_pool.tile([128, 2 * TCH], BF16, tag="ot",
                                               name="ot")
                nc.vector.tensor_copy(
                    ot_tiles[i][:, o * TCH:(o + 1) * TCH], ps[:])
                if o == 1:
                    nc.sync.dma_start(out[i * 128:(i + 1) * 128, :],
                                      ot_tiles.pop(i)[:])

            # ---- schedule ----
            def run_slot(h, cch, fillers, pre=0):
                """Emit head h's S/exp stream for chunk cch, spreading the
                filler thunks between S steps so the in-order PE queue always
                has ready work while ACT computes exp. The first `pre`
                fillers go ahead of the first S pair (covering repack/evict
                latency at slot starts)."""
                NP = 2 * cch + 2
                done = 0
                while done < pre:
                    fillers[done]()
                    done += 1
                for jp in range(NP):
                    do_S_pair(h, cch, jp)
                    want = max(done, (len(fillers) * (jp + 1)) // NP)
                    while done < want:
                        fillers[done]()
                        done += 1

            # PV(h) of chunk c is emitted two slots after its S stream:
            # pv0 -> slot2(c), pv1 -> slot3(c), pv2 -> slot0(c+1),
            # pv3 -> slot1(c+1); transposes + output projection for chunk c
            # land in slots 2/3 of chunk c+1. This balances filler work
            # across slots so no slot's PE stream is thinner than its ACT
            # exp load, and gives every PE consumer >= 1us of slack after
            # its DVE/DMA producer.
            for cch in range(NCH):
                i0 = 4 * cch
                p0 = i0 - 4  # prev chunk query-tile base
                last = cch == NCH - 1
                if cch == 0:
                    # first chunk chases the xT slab arrivals: 256-query
                    # projection halves per slab
                    do_proj(0, 0, halves=((0, 256), (256, 256)))
                    do_proj(2, 0, halves=((0, 256), (256, 256)))
                    s0 = [lambda i=i: do_v(i) for i in range(4)]
                    s1 = [lambda: do_proj(1, 0, halves=((0, 256),)),
                          lambda: do_proj(1, 0, halves=((256, 256),)),
                          lambda: do_proj(3, 0, halves=((0, 256),)),
                          lambda: do_proj(3, 0, halves=((256, 256),))]
                    s2 = [lambda i=i: do_pv(0, i) for i in range(4)]
                    s3 = [lambda i=i: do_pv(1, i) for i in range(4)]
                    pres = (0, 0, 0, 0)
                else:
                    do_proj(0, cch)
                    do_proj(2, cch)
                    s0 = [lambda: do_pv(2, p0 + 0),
                          lambda: do_v(i0 + 0),
                          lambda: do_pv(2, p0 + 1),
                          lambda: do_v(i0 + 1),
                          lambda: do_pv(2, p0 + 2),
                          lambda: do_v(i0 + 2),
                          lambda: do_pv(2, p0 + 3),
                          lambda: do_v(i0 + 3)]
                    s1 = [lambda: do_proj(1, cch),
                          lambda: do_pv(3, p0 + 0),
                          lambda: do_pv(3, p0 + 1),
                          lambda: do_proj(3, cch),
                          lambda: do_pv(3, p0 + 2),
                          lambda: do_pv(3, p0 + 3)]
                    s2 = [lambda: do_pv(0, i0 + 0),
                          lambda: do_T(p0 + 0),
                          lambda: do_pv(0, i0 + 1),
                          lambda: do_T(p0 + 1),
                          lambda: do_oproj(p0 + 0, 0),
                          lambda: do_pv(0, i0 + 2),
                          lambda: do_T(p0 + 2),
                          lambda: do_oproj(p0 + 0, 1),
                          lambda: do_pv(0, i0 + 3),
                          lambda: do_oproj(p0 + 1, 0),
                          lambda: do_T(p0 + 3),
                          lambda: do_oproj(p0 + 1, 1)]
                    s3 = [lambda: do_pv(1, i0 + 0),
                          lambda: do_oproj(p0 + 2, 0),
                          lambda: do_pv(1, i0 + 1),
                          lambda: do_oproj(p0 + 2, 1),
                          lambda: do_pv(1, i0 + 2),
                          lambda: do_oproj(p0 + 3, 0),
                          lambda: do_pv(1, i0 + 3),
                          lambda: do_oproj(p0 + 3, 1)]
                    if last:
                        # densify the final slot so the drain is shorter:
                        # head-2 PV for the first query tiles only needs
                        # slot-2's S pairs, which are already emitted
                        s3 = s3 + [lambda: do_pv(2, i0 + 0),
                                   lambda: do_pv(2, i0 + 1)]
                    pres = (0, 0, 0, 0)
                run_slot(0, cch, s0, pre=pres[0])
                run_slot(1, cch, s1, pre=pres[1])
                run_slot(2, cch, s2, pre=pres[2])
                run_slot(3, cch, s3, pre=pres[3])

            # drain: chunk 3's heads 2,3 PV, transposes, output projection
            i0 = 4 * (NCH - 1)
            for u in [
                lambda: do_pv(2, i0 + 2),
                lambda: do_pv(2, i0 + 3),
                lambda: do_pv(3, i0 + 0),
                lambda: do_pv(3, i0 + 1),
                lambda: do_T(i0 + 0),
                lambda: do_pv(3, i0 + 2),
                lambda: do_T(i0 + 1),
                lambda: do_oproj(i0 + 0, 0),
                lambda: do_pv(3, i0 + 3),
                lambda: do_T(i0 + 2),
                lambda: do_oproj(i0 + 0, 1),
                lambda: do_oproj(i0 + 1, 0),
                lambda: do_T(i0 + 3),
                lambda: do_oproj(i0 + 1, 1),
                lambda: do_oproj(i0 + 2, 0),
                lambda: do_oproj(i0 + 2, 1),
                lambda: do_oproj(i0 + 3, 0),
                lambda: do_oproj(i0 + 3, 1),
            ]:
                u()

    nc.compile()
    return nc


def _host_inputs(x, Wq, bq, Wk, bk, Wv, bv, Wp):
    """Slice + lay out per-core inputs (bf16 except the f32 biases)."""
    import ml_dtypes
    BF = ml_dtypes.bfloat16

    t2l = np.arange(128)[:, None]
    bl = np.arange(128)[None, :]
    tri = (t2l <= bl).astype(np.float32)  # [keys, queries] causal keep-mask

    def fold(a):
        # (kc*128, w) -> (128, kc*w): k-tile index moves into the free dim
        kc, w = a.shape[0] // 128, a.shape[1]
        return np.ascontiguousarray(
            a.reshape(kc, 128, w).transpose(1, 0, 2).reshape(128, kc * w))

    xTs = [np.ascontiguousarray(x[b].T).astype(BF) for b in range(B)]
    grp = []
    for g in range(GROUPS):
        hs = g * OS
        he = hs + OS
        # m-major wqk: column blocks in load order m0, m2, m1, m3
        wq_T = Wq[hs:he].T  # [C, 256]
        wk_T = Wk[hs:he].T
        mslab = {0: wq_T[:, 0:128], 1: wq_T[:, 128:256],
                 2: wk_T[:, 0:128], 3: wk_T[:, 128:256]}
        wqk_s = np.zeros((128, 4 * KC * 128), dtype=np.float32)
        for m in range(4):
            wqk_s[:, WQK_OFF[m]:WQK_OFF[m] + KC * 128] = fold(mslab[m])
        bqk = np.stack([bq[hs:hs + 128], bq[hs + 128:he],
                        bk[hs:hs + 128], bk[hs + 128:he]], axis=1)
        wv_aug = np.zeros((C, VW), dtype=np.float32)
        bv_aug = np.zeros((VW,), dtype=np.float32)
        for h in range(HPG):
            lo = h * (HD + 1)
            wv_aug[:, lo:lo + HD] = Wv[hs + h * HD:hs + (h + 1) * HD].T
            bv_aug[lo:lo + HD] = bv[hs + h * HD:hs + (h + 1) * HD]
            bv_aug[lo + HD] = 1.0
        wp_s = fold(np.ascontiguousarray(Wp[:, hs:he].T))
        smb = np.zeros((128, 644), dtype=np.float32)
        smb[:, 0:128] = tri
        smb[:, 128:256] = np.eye(128, dtype=np.float32)
        smb[0, 256:256 + VW] = bv_aug
        smb[0, 516:644] = 1.0
        grp.append({"wqk": wqk_s.astype(BF), "wv": fold(wv_aug).astype(BF),
                    "wp": wp_s.astype(BF), "smf": bqk.astype(np.float32),
                    "smb": smb.astype(BF)})

    in_maps = []
    for ci in range(N_CORES):
        b, g = divmod(ci, GROUPS)
        in_maps.append({"xT": xTs[b], **grp[g]})
    return in_maps


def kernel(x, Wq, bq, Wk, bk, Wv, bv, Wp, bp):
    x = np.asarray(x, dtype=np.float32)
    args = [np.asarray(a, dtype=np.float32)
            for a in (Wq, bq, Wk, bk, Wv, bv, Wp)]
    bp = np.asarray(bp, dtype=np.float32)

    if "nc" not in _CACHE:
        _CACHE["nc"] = _build()
    nc = _CACHE["nc"]

    in_maps = _host_inputs(x, *args)
    res = run_bass_kernel_spmd(nc, in_maps, list(range(N_CORES)))

    out = np.empty((B, T, C), dtype=np.float32)
    for b in range(B):
        acc = np.asarray(res.results[b * GROUPS]["out"], dtype=np.float32)
        for g in range(1, GROUPS):
            acc += np.asarray(res.results[b * GROUPS + g]["out"],
                              dtype=np.float32)
        out[b] = acc + bp
    return out
